# revision 55
# baseline (speedup 1.0000x reference)
"""BennaSynapse update kernel for Trainium2, SPMD over 8 NeuronCores.

Math: the (10, W1, W2) update-vector stack collapses into rank-1 structure.
With p = P_matrix[0], q = a1 @ W and scalar contractions s5, s67, s8:

    sum_i p[i] * uv[i] = e1^T v1 + a1^T v2 + 1^T v3 + cW * W
      v1 = -(p0 + p5*s5 + p7*s67) * a0 - p2 * e0
      v2 = p9 * a0 - (p1 + p6*s67 + p8*s8) * e0 - p9 * q
      v3 = -p4 * e0
      cW = -p3

    inChange = tanh(e1^T v1 + a1^T v2 + 1^T v3 + cW*W + bias)

The diffusion step is tridiagonal across the 5 chemicals with scalar
coefficients; out[i] = A_i*c[i-1] + B_i*c[i] + D_i*c[i+1] (+ E0*inChange
for i = 0).  The host pre-scales each stored plane by B_i
(dm[i] = B_i*c[i]) so every output needs only two fused
(in0*coef + in1) MACs:

    out[i] = A'_i*dm[i-1] + dm[i] + D'_i*dm[i+1],   A' = A/B_prev etc.

All plane traffic is fp16 (the diffusion tolerates it easily), halving
HBM bytes vs fp32: 6 planes in (5 chem + folded bias), 5 planes out.

Sharding: all (W1, W2) planes split along W1 rows across 8 cores; the
small vectors/scalars are computed on the host (they are the size-1
all-reduces in the reference) and passed per-core as tensors so the
compiled NEFF is input-value independent.

Device kernel per core (rows = 512, cols = 4096), per [128, 1024] tile:
  PE  : PSUM = lhs2^T @ rhs2 (rank-2 part) + I @ biasw   (fp16 in, f32 acc)
  ACT : t = tanh(PSUM)
  DVE : 6 fused MACs (planes 1, 3, 0)
  Pool: 3 fused MACs (planes 2, 4)
"""

from contextlib import ExitStack

import ml_dtypes
import numpy as np

import concourse.bass as bass
import concourse.tile as tile
from concourse import bacc, mybir
from concourse.bass_utils import run_bass_kernel_spmd


def _ensure_axon_ntff_hook():
    """The agent image's ``antenv`` lacks ``axon_hooks``; provide it so
    ``run_bass_kernel_spmd(trace=True)`` (BASS_TRACE=1) can profile
    instead of crashing on import. No-op when the module already exists
    or when libaxon_pjrt.so is unavailable."""
    try:
        from antenv.axon_hooks import get_axon_ntff_profile_hook  # noqa: F401
        return
    except ImportError:
        pass
    import contextlib
    import ctypes
    import sys
    import types

    so_path = "/opt/axon/libaxon_pjrt.so"
    hook = None
    try:
        lib = ctypes.CDLL(so_path)
        if hasattr(lib, "axon_start_nrt_profile"):
            lib.axon_start_nrt_profile.argtypes = [
                ctypes.POINTER(ctypes.c_int64),
                ctypes.c_size_t,
            ]
            lib.axon_start_nrt_profile.restype = ctypes.c_int64
            lib.axon_stop_nrt_profile.argtypes = [ctypes.c_char_p]
            lib.axon_stop_nrt_profile.restype = ctypes.c_int64

            @contextlib.contextmanager
            def _hook(output_dir, device_ids):
                import jax

                jax.devices()
                if device_ids:
                    ids = (ctypes.c_int64 * len(device_ids))(*device_ids)
                    rc = lib.axon_start_nrt_profile(ids, len(device_ids))
                else:
                    rc = lib.axon_start_nrt_profile(None, 0)
                if rc != 0:
                    raise RuntimeError(f"axon_start_nrt_profile rc={rc}")
                try:
                    yield
                finally:
                    n = lib.axon_stop_nrt_profile(str(output_dir).encode())
                    print(f"profile: {n} file(s) written to {output_dir}")

            hook = _hook
    except OSError:
        pass

    mod = types.ModuleType("antenv.axon_hooks")
    mod.get_axon_ntff_profile_hook = lambda: hook
    mod.set_axon_ntff_profile_hook = lambda h: None
    sys.modules["antenv.axon_hooks"] = mod
    try:
        import antenv

        antenv.axon_hooks = mod
    except ImportError:
        pass


_ensure_axon_ntff_hook()

F32 = mybir.dt.float32
BF16 = mybir.dt.bfloat16
F8 = mybir.dt.float8e4
NCORES = 8
L = 5
W1 = 4096
W2 = 4096
RPC = W1 // NCORES          # rows per core
NJT = RPC // 128            # partition tiles per core
NKQ = 4                     # column chunks per row tile
KQ = W2 // NKQ              # chunk width
HALF = 512                  # matmul free-dim / one fp32 PSUM bank

_CACHE = {}
LAST_RESULTS = None         # BassKernelResults of the most recent run


def _build_program():
    if "nc" in _CACHE:
        return _CACHE["nc"]

    nc = bacc.Bacc("TRN2", target_bir_lowering=False, debug=False)
    # Host-blocked input, row-major [row, kchunk, plane*KQ]:
    #   inblk (bf16): planes s0, s1, s2, s4 (prescaled chem)
    #   f8blk (fp8):  planes 64*s3, 64*biasw — these are only read by PE
    #     (via a (1/64)*I fp8 identity) and ACT (exact fp32 coefficients),
    #     so fp8 costs no DVE throughput and cuts input bytes by 17%.
    in_d = nc.declare_dram_parameter("inblk", [RPC, NKQ, 4 * KQ], BF16, isOutput=False)
    f8_d = nc.declare_dram_parameter("f8blk", [RPC, NKQ, 2 * KQ], F8, isOutput=False)
    l2_d = nc.declare_dram_parameter("lhs2", [2, RPC], BF16, isOutput=False)
    r2_d = nc.declare_dram_parameter("rhs2", [2, W2], BF16, isOutput=False)
    # Scaled identities for the PE plane-3 accumulation:
    # eyes[0]=A3'*I, eyes[1]=D3'*I (bf16)
    eye_d = nc.declare_dram_parameter("eyes", [2, 128, 128], BF16, isOutput=False)
    e64_d = nc.declare_dram_parameter("eye64", [128, 128], F8, isOutput=False)
    dco_d = nc.declare_dram_parameter("dcoef", [128, 16], F32, isOutput=False)
    # Output in the same blocked layout; host un-blocks after gather.
    o_d = nc.declare_dram_parameter("outblk", [RPC, NKQ, L * KQ], BF16, isOutput=True)

    TANH = mybir.ActivationFunctionType.Tanh
    MUL = mybir.AluOpType.mult
    ADD = mybir.AluOpType.add

    with ExitStack() as ctx:
        tc = ctx.enter_context(tile.TileContext(nc))
        cpool = ctx.enter_context(tc.tile_pool(name="const", bufs=1))
        inp = ctx.enter_context(tc.tile_pool(name="inp", bufs=10))
        icp = ctx.enter_context(tc.tile_pool(name="ic", bufs=2))
        tmp = ctx.enter_context(tc.tile_pool(name="tmp", bufs=2))
        outp = ctx.enter_context(tc.tile_pool(name="outp", bufs=3))
        psp = ctx.enter_context(
            tc.tile_pool(name="ps", bufs=4, space=bass.MemorySpace.PSUM)
        )

        l2 = cpool.tile([2, RPC], BF16)
        r2 = cpool.tile([2, W2], BF16)
        eyeA3 = cpool.tile([128, 128], BF16)
        eyeD3 = cpool.tile([128, 128], BF16)
        eye64 = cpool.tile([128, 128], F8)
        dco = cpool.tile([128, 16], F32)

        def emit_const_loads():
            nc.sync.dma_start(l2[:], l2_d[:])
            nc.sync.dma_start(r2[:], r2_d[:])
            nc.sync.dma_start(eyeA3[:], eye_d[0])
            nc.sync.dma_start(eyeD3[:], eye_d[1])
            nc.sync.dma_start(eye64[:], e64_d[:])
            nc.sync.dma_start(dco[:], dco_d[:])

        def sc(i):
            return dco[:, i : i + 1]

        def emit_loads(r0, q):
            # One 2D descriptor loads the 5 prescaled planes + folded bias.
            call = inp.tile([128, 4 * KQ], BF16, tag="call")
            nc.sync.dma_start(call[:], in_d[r0 : r0 + 128, q, :])
            f8t = inp.tile([128, 2 * KQ], F8, tag="f8t", name="f8t")
            nc.sync.dma_start(f8t[:], f8_d[r0 : r0 + 128, q, :])
            # planes: s0, s1, s2, s4 in bf16; 64*s3, 64*biasw in fp8
            ct = [call[:, m * KQ : (m + 1) * KQ] for m in range(4)]
            s38 = f8t[:, 0:KQ]
            bt = f8t[:, KQ : 2 * KQ]
            return bt, ct + [s38]

        def emit_compute(r0, q, bt, ct):
            s0, s1, s2, s4, s38 = ct
            k0 = q * KQ
            t = icp.tile([128, KQ], BF16, tag="t")
            # Same-lhsT matmuls grouped across PSUM tiles so the
            # stationary weights load once per group instead of per
            # matmul. pss = tanh argument, ps2/ps3 = plane accumulators.
            ps0 = psp.tile([128, HALF], F32, tag="ps", bufs=4)
            ps1 = psp.tile([128, HALF], F32, tag="ps", bufs=4)
            p30 = psp.tile([128, HALF], F32, tag="p3", bufs=4)
            p31 = psp.tile([128, HALF], F32, tag="p3", bufs=4)
            pss = [ps0, ps1]
            ps3 = [p30, p31]

            def half(ap, s):
                return ap[:, s * HALF : (s + 1) * HALF]

            for s in range(2):
                nc.tensor.matmul(
                    pss[s][:],
                    l2[:, r0 : r0 + 128],
                    r2[:, k0 + s * HALF : k0 + (s + 1) * HALF],
                    start=True,
                    stop=False,
                )
            # The fp8 (1/64)*I group adds the bias into the tanh arg and
            # starts plane 3 with its center term 64*s3/64. The bias-add
            # runs right after the rank-2 because it gates the tanh and
            # the whole plane-0 chain.
            for s in range(2):
                nc.tensor.matmul(
                    pss[s][:], eye64[:], half(bt, s), start=False, stop=True
                )
            for s in range(2):
                nc.tensor.matmul(
                    ps3[s][:], eye64[:], half(s38, s), start=True, stop=False
                )
            for s in range(2):
                nc.scalar.activation(half(t, s), pss[s][:], TANH)
            for s in range(2):
                nc.tensor.matmul(
                    ps3[s][:], eyeA3[:], half(s2, s), start=False, stop=False
                )
            for s in range(2):
                nc.tensor.matmul(
                    ps3[s][:], eyeD3[:], half(s4, s), start=False, stop=True
                )

            # dcoef columns (primed coefficients, see kernel()):
            # 0:D0'' 2:A1'' 3:D1' 8:A4'/64 9:D2'/64
            oall = outp.tile([128, L * KQ], BF16, tag="oall")
            out_sl = [oall[:, m * KQ : (m + 1) * KQ] for m in range(L)]

            # scalar_tensor_tensor has no 16-bit uop (runs 0.5x), so the
            # MACs decompose into tensor_scalar (4x bf16) + tensor_tensor
            # (2x bf16) on DVE, with scaled copies on ACT and one center
            # add on Pool to keep every engine under the per-chunk DMA
            # budget.
            def dve_ts(tag, in0, i):
                v = tmp.tile([128, KQ], BF16, tag=tag, name=tag)
                nc.vector.tensor_scalar(v[:], in0[:], sc(i), None, op0=MUL)
                return v

            def act_mul(tag, in0, i):
                m = tmp.tile([128, KQ], BF16, tag=tag, name=tag)
                nc.scalar.mul(m[:], in0[:], sc(i))
                return m

            TT = nc.vector.tensor_tensor
            # Planes 1-4 depend only on the chem load — compute and store
            # them first so the PE->ACT tanh latency gates only plane 0.
            m1 = act_mul("m1", s0, 2)                   # A1''*s0
            mA2 = act_mul("mA2", s1, 4)                 # A2'*s1
            m4 = act_mul("m4", s38, 8)                  # (A4'/64)*64*s3
            md2 = act_mul("md2", s38, 9)                # (D2'/64)*64*s3
            w1 = tmp.tile([128, KQ], BF16, tag="w1", name="w1")
            nc.gpsimd.tensor_tensor(w1[:], m1[:], s1[:], ADD)
            w2 = tmp.tile([128, KQ], BF16, tag="w2", name="w2")
            nc.gpsimd.tensor_tensor(w2[:], mA2[:], s2[:], ADD)
            v1 = dve_ts("v1", s2, 3)                    # D1'*s2
            TT(out_sl[1], v1[:], w1[:], ADD)
            TT(out_sl[2], w2[:], md2[:], ADD)
            for s in range(2):
                # PSUM -> SBUF bf16 drain of the PE-computed plane 3.
                nc.vector.tensor_scalar(
                    half(out_sl[3], s), ps3[s][:], 1.0, None, op0=MUL
                )
            TT(out_sl[4], m4[:], s4[:], ADD)
            nc.sync.dma_start(
                o_d[r0 : r0 + 128, q, KQ : L * KQ], oall[:, KQ : L * KQ]
            )

            u = dve_ts("u", s1, 0)                      # D0''*s1
            u0 = tmp.tile([128, KQ], BF16, tag="u0", name="u0")
            TT(u0[:], u[:], s0[:], ADD)
            # E0 is folded into the stored plane 0 / coefficients; the
            # host multiplies the returned plane 0 by E0.
            TT(out_sl[0], t[:], u0[:], ADD)
            nc.sync.dma_start(o_d[r0 : r0 + 128, q, 0:KQ], oall[:, 0:KQ])

        # Software-pipeline the DMA stream: issue loads LOOKAHEAD chunks
        # ahead of compute+stores so a store's semaphore wait on the Sync
        # engine never starves the DMA queue of load descriptors.
        chunks = [(jt * 128, q) for jt in range(NJT) for q in range(NKQ)]
        LOOKAHEAD = 9
        pending = {}
        for idx in range(len(chunks) + LOOKAHEAD):
            if idx < len(chunks):
                r0, q = chunks[idx]
                pending[idx] = emit_loads(r0, q)
            if idx == 0:
                # Consts issue after the first big load so the DMA engines
                # start on bulk data immediately.
                emit_const_loads()
            j = idx - LOOKAHEAD
            if j >= 0:
                r0, q = chunks[j]
                bt, ct = pending.pop(j)
                emit_compute(r0, q, bt, ct)

    nc.compile()
    _CACHE["nc"] = nc
    return nc


def _host_small(a0, a1, e0, e1, W, P_matrix, bias, C, G):
    """Small contractions + coefficient folding, on the host. These are
    the size-1 all-reduces of the reference plus folding the W and
    constant-row terms of the tanh argument into one bias plane."""
    p = P_matrix[0].astype(np.float64)
    a0v = a0[0].astype(np.float64)
    a1v = a1[0].astype(np.float64)
    e0v = e0[0].astype(np.float64)
    e1v = e1[0].astype(np.float64)

    q = a1.astype(np.float64) @ W.astype(np.float64)  # (1, W2)
    q = q[0]
    s5 = a1v.sum()
    s67 = float(q @ e0v)
    s8 = float(e1v @ (W.astype(np.float64) @ a0v))

    v1 = -(p[0] + p[5] * s5 + p[7] * s67) * a0v - p[2] * e0v
    v2 = p[9] * a0v - (p[1] + p[6] * s67 + p[8] * s8) * e0v - p[9] * q
    v3 = -p[4] * e0v
    cW = np.float32(-p[3])

    # tanh argument = e1^T v1 + a1^T v2 + biasw,  biasw = bias + cW*W + v3
    biasw = bias + cW * W
    biasw += v3.astype(np.float32)[None, :]

    Cd = C.astype(np.float64)
    Gd = G.astype(np.float64)
    # Diffusion: out[i] = A_i*c[i-1] + B_i*c[i] + D_i*c[i+1] (+E0*tanh for
    # i=0).
    A = np.zeros(L)
    B = np.zeros(L)
    D = np.zeros(L)
    B[0] = 1.0 - Gd[1] / Cd[0]
    D[0] = Gd[1] / Cd[0]
    E0 = 1.0 / Cd[0]
    for i in (1, 2, 3):
        A[i] = Gd[i - 1] / Cd[i]
        B[i] = 1.0 - (Gd[i - 1] + Gd[i + 1]) / Cd[i]
        D[i] = Gd[i + 1] / Cd[i]
    A[4] = Gd[3] / Cd[4]
    B[4] = 1.0 - (Gd[5] + Gd[3]) / Cd[4]

    return v1, v2, biasw, A, B, D, E0


def _numpy_fallback(chemical, biasw, e1, a1, v1, v2, A, B, D, E0):
    """Bit-for-bit-safe host path, used only if the B coefficients are
    too ill-conditioned for the prescaled-plane device kernel."""
    arg = (
        e1[0].astype(np.float64)[:, None] * v1[None, :]
        + a1[0].astype(np.float64)[:, None] * v2[None, :]
        + biasw.astype(np.float64)
    )
    t = np.tanh(arg)
    c = chemical.astype(np.float64)
    out = np.empty_like(c)
    out[0] = B[0] * c[0] + D[0] * c[1] + E0 * t
    for i in (1, 2, 3):
        out[i] = A[i] * c[i - 1] + B[i] * c[i] + D[i] * c[i + 1]
    out[4] = A[4] * c[3] + B[4] * c[4]
    return out.astype(np.float32)


def kernel(a0, a1, e0, e1, W, chemical, P_matrix, bias, C, G):
    global LAST_RESULTS
    a0, a1, e0, e1 = (np.asarray(x, np.float32) for x in (a0, a1, e0, e1))
    W = np.asarray(W, np.float32)
    chemical = np.asarray(chemical, np.float32)
    P_matrix = np.asarray(P_matrix, np.float32)
    bias = np.asarray(bias, np.float32)
    C = np.asarray(C, np.float32)
    G = np.asarray(G, np.float32)
    assert W.shape == (W1, W2) and chemical.shape == (L, W1, W2)

    v1, v2, biasw, A, B, D, E0 = _host_small(
        a0, a1, e0, e1, W, P_matrix, bias, C, G
    )

    # Primed coefficients for the prescaled planes s[i]. E0 is folded
    # into plane 0's stored scale (s0 = B0/E0*c0) and compensated in the
    # coefficients; the host multiplies the returned plane 0 by E0.
    coef = np.zeros(16, dtype=np.float64)
    ok = np.abs(B).min() >= 1e-2 and np.isfinite(E0) and abs(E0) > 1e-6
    if ok:
        coef[0] = D[0] / (B[1] * E0)          # D0''
        coef[2] = A[1] * E0 / B[0]            # A1''
        coef[3] = D[1] / B[2]                 # D1'
        coef[8] = A[4] / B[3] / 64.0          # A4' on the 64*s3 plane
        coef[9] = D[2] / B[3] / 64.0          # D2' on the 64*s3 plane
        a2p = A[2] / B[1]
        a3p = A[3] / B[2]
        d3p = D[3] / B[4]
        coef[4] = a2p
        coef[6] = a3p
        coef[7] = d3p
        ok = np.all(np.isfinite(coef)) and np.abs(coef).max() < 1e3
    if not ok:
        return _numpy_fallback(chemical, biasw, e1, a1, v1, v2, A, B, D, E0)

    dco = np.ascontiguousarray(
        np.broadcast_to(coef.astype(np.float32), (128, 16))
    )
    eyef = np.eye(128, dtype=np.float32)
    eyes = np.stack(
        [np.float32(a3p) * eyef, np.float32(d3p) * eyef]
    ).astype(ml_dtypes.bfloat16)
    eye64 = (eyef / 64.0).astype(ml_dtypes.float8_e4m3fn)

    # Blocked input layout [row, kchunk, plane, KQ]. bf16 planes are the
    # ones DVE touches (s0, s1, s2, s4); s3 and the folded bias ship as
    # 64x-scaled fp8 (PE/ACT-only consumers, exact (1/64) descale).
    pre = np.array([B[0] / E0, B[1], B[2], B[4]], dtype=np.float32)
    inblk = np.empty((W1, NKQ, 4, KQ), dtype=ml_dtypes.bfloat16)
    for j, k in enumerate((0, 1, 2, 4)):
        inblk[:, :, j, :] = (chemical[k] * pre[j]).reshape(W1, NKQ, KQ)
    inblk = inblk.reshape(W1, NKQ, 4 * KQ)
    f8blk = np.empty((W1, NKQ, 2, KQ), dtype=ml_dtypes.float8_e4m3fn)
    f8blk[:, :, 0, :] = (chemical[3] * np.float32(64.0 * B[3])).reshape(
        W1, NKQ, KQ
    )
    f8blk[:, :, 1, :] = (biasw * np.float32(64.0)).reshape(W1, NKQ, KQ)
    f8blk = f8blk.reshape(W1, NKQ, 2 * KQ)

    rhs2 = np.stack([v1, v2]).astype(ml_dtypes.bfloat16)

    in_maps = []
    for c in range(NCORES):
        rs = slice(c * RPC, (c + 1) * RPC)
        lhs2 = np.ascontiguousarray(
            np.stack([e1[0, rs], a1[0, rs]]).astype(ml_dtypes.bfloat16)
        )
        in_maps.append(
            dict(
                inblk=inblk[rs],
                f8blk=f8blk[rs],
                lhs2=lhs2,
                rhs2=rhs2,
                eyes=eyes,
                eye64=eye64,
                dcoef=dco,
            )
        )

    nc = _build_program()
    LAST_RESULTS = run_bass_kernel_spmd(nc, in_maps, list(range(NCORES)))
    res = LAST_RESULTS.results

    outblk = np.concatenate(
        [res[c]["outblk"].reshape(RPC, NKQ, L, KQ) for c in range(NCORES)], axis=0
    )
    out = np.ascontiguousarray(
        outblk.transpose(2, 0, 1, 3).reshape(L, W1, W2).astype(np.float32)
    )
    out[0] *= np.float32(E0)   # undo the plane-0 E0 fold
    return out


# revision 56
# speedup vs baseline: 1.3603x; 1.3603x over previous
"""BennaSynapse update kernel for Trainium2, SPMD over 8 NeuronCores.

Math: the (10, W1, W2) update-vector stack collapses into rank-1 structure.
With p = P_matrix[0], q = a1 @ W and scalar contractions s5, s67, s8:

    sum_i p[i] * uv[i] = e1^T v1 + a1^T v2 + 1^T v3 + cW * W
      v1 = -(p0 + p5*s5 + p7*s67) * a0 - p2 * e0
      v2 = p9 * a0 - (p1 + p6*s67 + p8*s8) * e0 - p9 * q
      v3 = -p4 * e0
      cW = -p3

    inChange = tanh(e1^T v1 + a1^T v2 + 1^T v3 + cW*W + bias)

The diffusion step is tridiagonal across the 5 chemicals with scalar
coefficients; out[i] = A_i*c[i-1] + B_i*c[i] + D_i*c[i+1] (+ E0*inChange
for i = 0).  The host pre-scales each stored plane by B_i
(dm[i] = B_i*c[i]) so every output needs only two fused
(in0*coef + in1) MACs:

    out[i] = A'_i*dm[i-1] + dm[i] + D'_i*dm[i+1],   A' = A/B_prev etc.

All plane traffic is fp16 (the diffusion tolerates it easily), halving
HBM bytes vs fp32: 6 planes in (5 chem + folded bias), 5 planes out.

Sharding: all (W1, W2) planes split along W1 rows across 8 cores; the
small vectors/scalars are computed on the host (they are the size-1
all-reduces in the reference) and passed per-core as tensors so the
compiled NEFF is input-value independent.

Device kernel per core (rows = 512, cols = 4096), per [128, 1024] tile:
  PE  : PSUM = lhs2^T @ rhs2 (rank-2 part) + I @ biasw   (fp16 in, f32 acc)
  ACT : t = tanh(PSUM)
  DVE : 6 fused MACs (planes 1, 3, 0)
  Pool: 3 fused MACs (planes 2, 4)
"""

from contextlib import ExitStack

import ml_dtypes
import numpy as np

import concourse.bass as bass
import concourse.tile as tile
from concourse import bacc, mybir
from concourse.bass_utils import run_bass_kernel_spmd


def _ensure_axon_ntff_hook():
    """The agent image's ``antenv`` lacks ``axon_hooks``; provide it so
    ``run_bass_kernel_spmd(trace=True)`` (BASS_TRACE=1) can profile
    instead of crashing on import. No-op when the module already exists
    or when libaxon_pjrt.so is unavailable."""
    try:
        from antenv.axon_hooks import get_axon_ntff_profile_hook  # noqa: F401
        return
    except ImportError:
        pass
    import contextlib
    import ctypes
    import sys
    import types

    so_path = "/opt/axon/libaxon_pjrt.so"
    hook = None
    try:
        lib = ctypes.CDLL(so_path)
        if hasattr(lib, "axon_start_nrt_profile"):
            lib.axon_start_nrt_profile.argtypes = [
                ctypes.POINTER(ctypes.c_int64),
                ctypes.c_size_t,
            ]
            lib.axon_start_nrt_profile.restype = ctypes.c_int64
            lib.axon_stop_nrt_profile.argtypes = [ctypes.c_char_p]
            lib.axon_stop_nrt_profile.restype = ctypes.c_int64

            @contextlib.contextmanager
            def _hook(output_dir, device_ids):
                import jax

                jax.devices()
                if device_ids:
                    ids = (ctypes.c_int64 * len(device_ids))(*device_ids)
                    rc = lib.axon_start_nrt_profile(ids, len(device_ids))
                else:
                    rc = lib.axon_start_nrt_profile(None, 0)
                if rc != 0:
                    raise RuntimeError(f"axon_start_nrt_profile rc={rc}")
                try:
                    yield
                finally:
                    n = lib.axon_stop_nrt_profile(str(output_dir).encode())
                    print(f"profile: {n} file(s) written to {output_dir}")

            hook = _hook
    except OSError:
        pass

    mod = types.ModuleType("antenv.axon_hooks")
    mod.get_axon_ntff_profile_hook = lambda: hook
    mod.set_axon_ntff_profile_hook = lambda h: None
    sys.modules["antenv.axon_hooks"] = mod
    try:
        import antenv

        antenv.axon_hooks = mod
    except ImportError:
        pass


_ensure_axon_ntff_hook()

F32 = mybir.dt.float32
BF16 = mybir.dt.bfloat16
F8 = mybir.dt.float8e4
NCORES = 8
L = 5
W1 = 4096
W2 = 4096
RPC = W1 // NCORES          # rows per core
NJT = RPC // 128            # partition tiles per core
NKQ = 4                     # column chunks per row tile
KQ = W2 // NKQ              # chunk width
HALF = 512                  # matmul free-dim / one fp32 PSUM bank

_CACHE = {}
LAST_RESULTS = None         # BassKernelResults of the most recent run


def _build_program():
    if "nc" in _CACHE:
        return _CACHE["nc"]

    nc = bacc.Bacc("TRN2", target_bir_lowering=False, debug=False)
    # Host-blocked input, row-major [row, kchunk, plane*KQ]:
    #   inblk (bf16): planes s0, s1, s2, s4 (prescaled chem)
    #   f8blk (fp8):  planes 64*s3, 64*biasw — these are only read by PE
    #     (via a (1/64)*I fp8 identity) and ACT (exact fp32 coefficients),
    #     so fp8 costs no DVE throughput and cuts input bytes by 17%.
    in_d = nc.declare_dram_parameter("inblk", [RPC, NKQ, 4 * KQ], BF16, isOutput=False)
    f8_d = nc.declare_dram_parameter("f8blk", [RPC, NKQ, 2 * KQ], F8, isOutput=False)
    l2_d = nc.declare_dram_parameter("lhs2", [2, RPC], BF16, isOutput=False)
    r2_d = nc.declare_dram_parameter("rhs2", [2, W2], BF16, isOutput=False)
    # Scaled identities for the PE plane accumulation:
    # eyes[0]=I, eyes[1]=A2'*I, eyes[2]=A3'*I, eyes[3]=D3'*I (bf16)
    eye_d = nc.declare_dram_parameter("eyes", [4, 128, 128], BF16, isOutput=False)
    e64_d = nc.declare_dram_parameter("eye64", [128, 128], F8, isOutput=False)
    dco_d = nc.declare_dram_parameter("dcoef", [128, 16], F32, isOutput=False)
    # Output in the same blocked layout; host un-blocks after gather.
    o_d = nc.declare_dram_parameter("outblk", [RPC, NKQ, L * KQ], BF16, isOutput=True)

    TANH = mybir.ActivationFunctionType.Tanh
    MUL = mybir.AluOpType.mult
    ADD = mybir.AluOpType.add

    with ExitStack() as ctx:
        tc = ctx.enter_context(tile.TileContext(nc))
        cpool = ctx.enter_context(tc.tile_pool(name="const", bufs=1))
        inp = ctx.enter_context(tc.tile_pool(name="inp", bufs=10))
        icp = ctx.enter_context(tc.tile_pool(name="ic", bufs=2))
        tmp = ctx.enter_context(tc.tile_pool(name="tmp", bufs=2))
        outp = ctx.enter_context(tc.tile_pool(name="outp", bufs=3))
        psp = ctx.enter_context(
            tc.tile_pool(name="ps", bufs=4, space=bass.MemorySpace.PSUM)
        )

        l2 = cpool.tile([2, RPC], BF16)
        r2 = cpool.tile([2, W2], BF16)
        eye = cpool.tile([128, 128], BF16)
        eyeA2 = cpool.tile([128, 128], BF16)
        eyeA3 = cpool.tile([128, 128], BF16)
        eyeD3 = cpool.tile([128, 128], BF16)
        eye64 = cpool.tile([128, 128], F8)
        dco = cpool.tile([128, 16], F32)
        wu = cpool.tile([128, 128], BF16)

        def emit_const_loads():
            nc.sync.dma_start(l2[:], l2_d[:])
            nc.sync.dma_start(r2[:], r2_d[:])
            nc.sync.dma_start(eye[:], eye_d[0])
            nc.sync.dma_start(eyeA2[:], eye_d[1])
            nc.sync.dma_start(eyeA3[:], eye_d[2])
            nc.sync.dma_start(eyeD3[:], eye_d[3])
            nc.sync.dma_start(eye64[:], e64_d[:])
            nc.sync.dma_start(dco[:], dco_d[:])

        def emit_pe_warmup():
            # Back-to-back matmuls during the (PE-idle) load ramp push
            # the HAM activity monitor into the 2.4 GHz state before the
            # first real chunk computes; cold runs otherwise stay at
            # 1.2 GHz and become PE-bound.
            nc.vector.memset(wu[:], 0.0)
            wps = psp.tile([128, HALF], F32, tag="ps", bufs=4)
            for i in range(30):
                nc.tensor.matmul(
                    wps[:, 0:128], wu[:], wu[:],
                    start=(i == 0), stop=(i == 29),
                )

        def sc(i):
            return dco[:, i : i + 1]

        def emit_loads(r0, q):
            # One 2D descriptor loads the 5 prescaled planes + folded bias.
            call = inp.tile([128, 4 * KQ], BF16, tag="call")
            nc.sync.dma_start(call[:], in_d[r0 : r0 + 128, q, :])
            f8t = inp.tile([128, 2 * KQ], F8, tag="f8t", name="f8t")
            nc.sync.dma_start(f8t[:], f8_d[r0 : r0 + 128, q, :])
            # planes: s0, s1, s2, s4 in bf16; 64*s3, 64*biasw in fp8
            ct = [call[:, m * KQ : (m + 1) * KQ] for m in range(4)]
            s38 = f8t[:, 0:KQ]
            bt = f8t[:, KQ : 2 * KQ]
            return bt, ct + [s38]

        def emit_compute(r0, q, bt, ct):
            s0, s1, s2, s4, s38 = ct
            k0 = q * KQ
            t = icp.tile([128, KQ], BF16, tag="t")
            # Same-lhsT matmuls grouped across PSUM tiles so the
            # stationary weights load once per group instead of per
            # matmul. pss = tanh argument, ps2/ps3 = plane accumulators.
            ps0 = psp.tile([128, HALF], F32, tag="ps", bufs=4)
            ps1 = psp.tile([128, HALF], F32, tag="ps", bufs=4)
            p20 = psp.tile([128, HALF], F32, tag="p2", bufs=2)
            p21 = psp.tile([128, HALF], F32, tag="p2", bufs=2)
            p30 = psp.tile([128, HALF], F32, tag="p3", bufs=2)
            p31 = psp.tile([128, HALF], F32, tag="p3", bufs=2)
            pss = [ps0, ps1]
            ps2 = [p20, p21]
            ps3 = [p30, p31]

            def half(ap, s):
                return ap[:, s * HALF : (s + 1) * HALF]

            for s in range(2):
                nc.tensor.matmul(
                    pss[s][:],
                    l2[:, r0 : r0 + 128],
                    r2[:, k0 + s * HALF : k0 + (s + 1) * HALF],
                    start=True,
                    stop=False,
                )
            # The fp8 (1/64)*I group adds the bias into the tanh arg and
            # starts plane 3 with its center term 64*s3/64. The bias-add
            # runs right after the rank-2 because it gates the tanh and
            # the whole plane-0 chain.
            for s in range(2):
                nc.tensor.matmul(
                    pss[s][:], eye64[:], half(bt, s), start=False, stop=True
                )
            for s in range(2):
                nc.tensor.matmul(
                    ps3[s][:], eye64[:], half(s38, s), start=True, stop=False
                )
            for s in range(2):
                nc.scalar.activation(half(t, s), pss[s][:], TANH)
            # Plane 2 on PE: I @ s2 + A2'*I @ s1 (its D2'*s3 term joins
            # at drain time from an ACT-scaled copy of the fp8 s3).
            for s in range(2):
                nc.tensor.matmul(
                    ps2[s][:], eye[:], half(s2, s), start=True, stop=False
                )
            for s in range(2):
                nc.tensor.matmul(
                    ps2[s][:], eyeA2[:], half(s1, s), start=False, stop=True
                )
            for s in range(2):
                nc.tensor.matmul(
                    ps3[s][:], eyeA3[:], half(s2, s), start=False, stop=False
                )
            for s in range(2):
                nc.tensor.matmul(
                    ps3[s][:], eyeD3[:], half(s4, s), start=False, stop=True
                )

            # dcoef columns (primed coefficients, see kernel()):
            # 0:D0'' 2:A1'' 3:D1' 8:A4'/64 9:D2'/64
            oall = outp.tile([128, L * KQ], BF16, tag="oall")
            out_sl = [oall[:, m * KQ : (m + 1) * KQ] for m in range(L)]

            # scalar_tensor_tensor has no 16-bit uop (runs 0.5x), so the
            # MACs decompose into tensor_scalar (4x bf16) + tensor_tensor
            # (2x bf16) on DVE, with scaled copies on ACT and one center
            # add on Pool to keep every engine under the per-chunk DMA
            # budget.
            def dve_ts(tag, in0, i):
                v = tmp.tile([128, KQ], BF16, tag=tag, name=tag)
                nc.vector.tensor_scalar(v[:], in0[:], sc(i), None, op0=MUL)
                return v

            def act_mul(tag, in0, i):
                m = tmp.tile([128, KQ], BF16, tag=tag, name=tag)
                nc.scalar.mul(m[:], in0[:], sc(i))
                return m

            TT = nc.vector.tensor_tensor
            # Planes 1-4 depend only on the chem load — compute and store
            # them first so the PE->ACT tanh latency gates only plane 0.
            m1 = act_mul("m1", s0, 2)                   # A1''*s0
            m4 = act_mul("m4", s38, 8)                  # (A4'/64)*64*s3
            md2 = act_mul("md2", s38, 9)                # (D2'/64)*64*s3
            w1 = tmp.tile([128, KQ], BF16, tag="w1", name="w1")
            nc.gpsimd.tensor_tensor(w1[:], m1[:], s1[:], ADD)
            v1 = dve_ts("v1", s2, 3)                    # D1'*s2
            TT(out_sl[1], v1[:], w1[:], ADD)
            for s in range(2):
                # PSUM -> SBUF bf16 drains of the PE-computed planes;
                # plane 2's drain folds in its D2'*s3 term.
                TT(half(out_sl[2], s), ps2[s][:], half(md2, s), ADD)
            for s in range(2):
                nc.vector.tensor_scalar(
                    half(out_sl[3], s), ps3[s][:], 1.0, None, op0=MUL
                )
            TT(out_sl[4], m4[:], s4[:], ADD)
            nc.sync.dma_start(
                o_d[r0 : r0 + 128, q, KQ : L * KQ], oall[:, KQ : L * KQ]
            )

            u = dve_ts("u", s1, 0)                      # D0''*s1
            u0 = tmp.tile([128, KQ], BF16, tag="u0", name="u0")
            TT(u0[:], u[:], s0[:], ADD)
            # E0 is folded into the stored plane 0 / coefficients; the
            # host multiplies the returned plane 0 by E0.
            TT(out_sl[0], t[:], u0[:], ADD)
            nc.sync.dma_start(o_d[r0 : r0 + 128, q, 0:KQ], oall[:, 0:KQ])

        # Software-pipeline the DMA stream: issue loads LOOKAHEAD chunks
        # ahead of compute+stores so a store's semaphore wait on the Sync
        # engine never starves the DMA queue of load descriptors.
        chunks = [(jt * 128, q) for jt in range(NJT) for q in range(NKQ)]
        LOOKAHEAD = 9
        pending = {}
        for idx in range(len(chunks) + LOOKAHEAD):
            if idx < len(chunks):
                r0, q = chunks[idx]
                pending[idx] = emit_loads(r0, q)
            if idx == 0:
                # Consts issue after the first big load so the DMA engines
                # start on bulk data immediately.
                emit_const_loads()
                emit_pe_warmup()
            j = idx - LOOKAHEAD
            if j >= 0:
                r0, q = chunks[j]
                bt, ct = pending.pop(j)
                emit_compute(r0, q, bt, ct)

    nc.compile()
    _CACHE["nc"] = nc
    return nc


def _host_small(a0, a1, e0, e1, W, P_matrix, bias, C, G):
    """Small contractions + coefficient folding, on the host. These are
    the size-1 all-reduces of the reference plus folding the W and
    constant-row terms of the tanh argument into one bias plane."""
    p = P_matrix[0].astype(np.float64)
    a0v = a0[0].astype(np.float64)
    a1v = a1[0].astype(np.float64)
    e0v = e0[0].astype(np.float64)
    e1v = e1[0].astype(np.float64)

    q = a1.astype(np.float64) @ W.astype(np.float64)  # (1, W2)
    q = q[0]
    s5 = a1v.sum()
    s67 = float(q @ e0v)
    s8 = float(e1v @ (W.astype(np.float64) @ a0v))

    v1 = -(p[0] + p[5] * s5 + p[7] * s67) * a0v - p[2] * e0v
    v2 = p[9] * a0v - (p[1] + p[6] * s67 + p[8] * s8) * e0v - p[9] * q
    v3 = -p[4] * e0v
    cW = np.float32(-p[3])

    # tanh argument = e1^T v1 + a1^T v2 + biasw,  biasw = bias + cW*W + v3
    biasw = bias + cW * W
    biasw += v3.astype(np.float32)[None, :]

    Cd = C.astype(np.float64)
    Gd = G.astype(np.float64)
    # Diffusion: out[i] = A_i*c[i-1] + B_i*c[i] + D_i*c[i+1] (+E0*tanh for
    # i=0).
    A = np.zeros(L)
    B = np.zeros(L)
    D = np.zeros(L)
    B[0] = 1.0 - Gd[1] / Cd[0]
    D[0] = Gd[1] / Cd[0]
    E0 = 1.0 / Cd[0]
    for i in (1, 2, 3):
        A[i] = Gd[i - 1] / Cd[i]
        B[i] = 1.0 - (Gd[i - 1] + Gd[i + 1]) / Cd[i]
        D[i] = Gd[i + 1] / Cd[i]
    A[4] = Gd[3] / Cd[4]
    B[4] = 1.0 - (Gd[5] + Gd[3]) / Cd[4]

    return v1, v2, biasw, A, B, D, E0


def _numpy_fallback(chemical, biasw, e1, a1, v1, v2, A, B, D, E0):
    """Bit-for-bit-safe host path, used only if the B coefficients are
    too ill-conditioned for the prescaled-plane device kernel."""
    arg = (
        e1[0].astype(np.float64)[:, None] * v1[None, :]
        + a1[0].astype(np.float64)[:, None] * v2[None, :]
        + biasw.astype(np.float64)
    )
    t = np.tanh(arg)
    c = chemical.astype(np.float64)
    out = np.empty_like(c)
    out[0] = B[0] * c[0] + D[0] * c[1] + E0 * t
    for i in (1, 2, 3):
        out[i] = A[i] * c[i - 1] + B[i] * c[i] + D[i] * c[i + 1]
    out[4] = A[4] * c[3] + B[4] * c[4]
    return out.astype(np.float32)


def kernel(a0, a1, e0, e1, W, chemical, P_matrix, bias, C, G):
    global LAST_RESULTS
    a0, a1, e0, e1 = (np.asarray(x, np.float32) for x in (a0, a1, e0, e1))
    W = np.asarray(W, np.float32)
    chemical = np.asarray(chemical, np.float32)
    P_matrix = np.asarray(P_matrix, np.float32)
    bias = np.asarray(bias, np.float32)
    C = np.asarray(C, np.float32)
    G = np.asarray(G, np.float32)
    assert W.shape == (W1, W2) and chemical.shape == (L, W1, W2)

    v1, v2, biasw, A, B, D, E0 = _host_small(
        a0, a1, e0, e1, W, P_matrix, bias, C, G
    )

    # Primed coefficients for the prescaled planes s[i]. E0 is folded
    # into plane 0's stored scale (s0 = B0/E0*c0) and compensated in the
    # coefficients; the host multiplies the returned plane 0 by E0.
    coef = np.zeros(16, dtype=np.float64)
    ok = np.abs(B).min() >= 1e-2 and np.isfinite(E0) and abs(E0) > 1e-6
    if ok:
        coef[0] = D[0] / (B[1] * E0)          # D0''
        coef[2] = A[1] * E0 / B[0]            # A1''
        coef[3] = D[1] / B[2]                 # D1'
        coef[8] = A[4] / B[3] / 64.0          # A4' on the 64*s3 plane
        coef[9] = D[2] / B[3] / 64.0          # D2' on the 64*s3 plane
        a2p = A[2] / B[1]
        a3p = A[3] / B[2]
        d3p = D[3] / B[4]
        coef[4] = a2p
        coef[6] = a3p
        coef[7] = d3p
        ok = np.all(np.isfinite(coef)) and np.abs(coef).max() < 1e3
    if not ok:
        return _numpy_fallback(chemical, biasw, e1, a1, v1, v2, A, B, D, E0)

    dco = np.ascontiguousarray(
        np.broadcast_to(coef.astype(np.float32), (128, 16))
    )
    eyef = np.eye(128, dtype=np.float32)
    eyes = np.stack(
        [
            eyef,
            np.float32(a2p) * eyef,
            np.float32(a3p) * eyef,
            np.float32(d3p) * eyef,
        ]
    ).astype(ml_dtypes.bfloat16)
    eye64 = (eyef / 64.0).astype(ml_dtypes.float8_e4m3fn)

    # Blocked input layout [row, kchunk, plane, KQ]. bf16 planes are the
    # ones DVE touches (s0, s1, s2, s4); s3 and the folded bias ship as
    # 64x-scaled fp8 (PE/ACT-only consumers, exact (1/64) descale).
    pre = np.array([B[0] / E0, B[1], B[2], B[4]], dtype=np.float32)
    inblk = np.empty((W1, NKQ, 4, KQ), dtype=ml_dtypes.bfloat16)
    for j, k in enumerate((0, 1, 2, 4)):
        inblk[:, :, j, :] = (chemical[k] * pre[j]).reshape(W1, NKQ, KQ)
    inblk = inblk.reshape(W1, NKQ, 4 * KQ)
    f8blk = np.empty((W1, NKQ, 2, KQ), dtype=ml_dtypes.float8_e4m3fn)
    f8blk[:, :, 0, :] = (chemical[3] * np.float32(64.0 * B[3])).reshape(
        W1, NKQ, KQ
    )
    f8blk[:, :, 1, :] = (biasw * np.float32(64.0)).reshape(W1, NKQ, KQ)
    f8blk = f8blk.reshape(W1, NKQ, 2 * KQ)

    rhs2 = np.stack([v1, v2]).astype(ml_dtypes.bfloat16)

    in_maps = []
    for c in range(NCORES):
        rs = slice(c * RPC, (c + 1) * RPC)
        lhs2 = np.ascontiguousarray(
            np.stack([e1[0, rs], a1[0, rs]]).astype(ml_dtypes.bfloat16)
        )
        in_maps.append(
            dict(
                inblk=inblk[rs],
                f8blk=f8blk[rs],
                lhs2=lhs2,
                rhs2=rhs2,
                eyes=eyes,
                eye64=eye64,
                dcoef=dco,
            )
        )

    nc = _build_program()
    LAST_RESULTS = run_bass_kernel_spmd(nc, in_maps, list(range(NCORES)))
    res = LAST_RESULTS.results

    outblk = np.concatenate(
        [res[c]["outblk"].reshape(RPC, NKQ, L, KQ) for c in range(NCORES)], axis=0
    )
    out = np.ascontiguousarray(
        outblk.transpose(2, 0, 1, 3).reshape(L, W1, W2).astype(np.float32)
    )
    out[0] *= np.float32(E0)   # undo the plane-0 E0 fold
    return out
